# revision 49
# baseline (speedup 1.0000x reference)
import sys, os
import numpy as np

for p in ("/opt/trn_rl_repo",):
    if p not in sys.path:
        sys.path.insert(0, p)

NC_CAP, DC, ROUT, EPS = 16, 32, 3, 1e-7
B, S, DIN, O = 256, 512, 256, 512   # full problem;  O = NC_CAP*DC
NCORES = 8
BPC = B // NCORES                   # 32 batches per core
NB = 8                              # batches per routing group (free axis = 16*NB)
NG = BPC // NB                      # 4 groups per core

LAST_RESULTS = None


def _split_bir_waits(bir_json):
    """Reduce every instruction to at most one sync-wait command.

    The walrus build in this container rejects instructions with more than
    one sync wait. Two rewrites, semantics preserving:
    1. Drop waits on the instruction's own engine semaphore when another
       wait remains — engines retire instructions in order, so a wait on
       an earlier same-engine instruction is implied by program order.
    2. Split any remaining extra waits onto preceding same-engine Drain
       carriers (AND of conditions == sequential waits).
    """
    import json
    d = json.loads(bir_json)
    engs = ("PE", "DVE", "Activation", "Pool", "SP")
    for f in d["functions"]:
        for bb in f["blocks"]:
            out = []
            for ins in bb["instructions"]:
                si = ins.get("sync_info")
                if si:
                    waits = si.get("on_wait") or []
                    if len(waits) > 1:
                        eng = ins["engine"]
                        own = [w for w in waits
                               if w.get("ant_name", "").startswith(eng + "_")
                               and eng in engs]
                        foreign = [w for w in waits if w not in own]
                        if foreign:
                            waits = foreign
                        else:
                            waits = [waits[-1]]
                    if len(waits) > 1:
                        for i, w in enumerate(waits[:-1]):
                            out.append({
                                "debug": ins.get("debug", 0),
                                "engine": ins["engine"],
                                "ins": [], "outs": [],
                                "name": f"{ins['name']}_w{i}",
                                "opcode": "Drain",
                                "sync_info": {"on_update": [], "on_wait": [w]},
                            })
                        waits = [waits[-1]]
                    si["on_wait"] = waits
                out.append(ins)
            bb["instructions"] = out
    return json.dumps(d).encode()


_COMPILE_PATCHED = False


def _install_compile_patch():
    global _COMPILE_PATCHED
    if _COMPILE_PATCHED:
        return
    import concourse.bass_utils as _bu
    import concourse.bass2jax as _b2j
    _orig = _bu.compile_bir_kernel

    def _patched(bir_json, tmpdir, neff_name="file.neff", **kw):
        return _orig(_split_bir_waits(bir_json), tmpdir, neff_name=neff_name, **kw)

    _bu.compile_bir_kernel = _patched
    _b2j.compile_bir_kernel = _patched
    # Make trace=True (BASS_TRACE) usable: this image lacks antenv.axon_hooks,
    # so synthesize it and register the NTFF hook via the libaxon C ABI.
    try:
        import types
        import antenv
        try:
            from antenv.axon_hooks import get_axon_ntff_profile_hook  # noqa
        except ImportError:
            _m = types.ModuleType("antenv.axon_hooks")
            _h = [None]
            _m.set_axon_ntff_profile_hook = lambda hook: _h.__setitem__(0, hook)
            _m.get_axon_ntff_profile_hook = lambda: _h[0]
            sys.modules["antenv.axon_hooks"] = _m
            antenv.axon_hooks = _m
            try:
                if "/root/.axon_site" not in sys.path:
                    sys.path.insert(0, "/root/.axon_site")
                from trn_agent_boot.trn_boot import _ntff_profile_via_ctypes
                _m.set_axon_ntff_profile_hook(
                    _ntff_profile_via_ctypes("/opt/axon/libaxon_pjrt.so"))
            except Exception:
                pass
        _bu.upload_artifacts = lambda d: d  # no artifact store in this image
    except Exception:
        pass
    if os.environ.get("BASS_LDW_OPT"):
        _orig_run = _bu.run_command

        def _patched_run(argv, **kw):
            argv = [a.replace("--enable-ldw-opt=false", "--enable-ldw-opt=true")
                    for a in argv]
            return _orig_run(argv, **kw)

        _bu.run_command = _patched_run
    _COMPILE_PATCHED = True


def _kernel_numpy(u_vecs, W):
    u = u_vecs.astype(np.float32)
    w = W[0].astype(np.float32)
    uh = np.einsum('bsi,io->bso', u, w)
    uh = uh.reshape(B, S, NC_CAP, DC).transpose(0, 2, 1, 3)
    b = np.zeros((B, NC_CAP, S), dtype=np.float32)
    out = None
    for i in range(ROUT):
        m = b.max(axis=1, keepdims=True)
        e = np.exp(b - m)
        c = e / e.sum(axis=1, keepdims=True)
        o = np.einsum('bni,bnid->bnd', c, uh)
        out = o / np.sqrt((o * o).sum(-1, keepdims=True) + EPS)
        if i < ROUT - 1:
            b = np.einsum('bnd,bnid->bni', out, uh)
    return out.astype(np.float32)


def _const_blocks():
    """Host-built constant blocks DMA'd in as extra inputs."""
    import ml_dtypes
    bf16 = ml_dtypes.bfloat16
    # CB (bf16) [128, 289]: ident(128) | c0(128) | psel(32) | ones_col(1)
    cb = np.zeros((128, 289), dtype=np.float32)
    cb[:, 0:128] = np.eye(128, dtype=np.float32)
    cb[:, 128:256] = 1.0 / 16.0
    r = np.arange(128)
    cb[:, 256:288] = (r[:, None] % 32 == np.arange(32)[None, :]).astype(np.float32)
    cb[:, 288] = 1.0
    # CF (f32) [128, 640]: masks 4x128 | ones block 128
    cf = np.zeros((128, 640), dtype=np.float32)
    n_of_col = np.arange(128) % 16
    for ot in range(4):
        m = ((4 * ot + r[:, None] // 32) == n_of_col[None, :]).astype(np.float32)
        cf[:, 128 * ot:128 * (ot + 1)] = m
    cf[:, 512:640] = 1.0
    return cb.astype(bf16), cf


def _build_bass(bpc=BPC, repeat=1, phase="full"):
    import concourse.bass as bass
    import concourse.tile as tile
    from concourse import mybir
    from contextlib import ExitStack

    f32, bf16 = mybir.dt.float32, mybir.dt.bfloat16
    AF = mybir.ActivationFunctionType
    ng = bpc // NB

    nc = bass.Bass()
    u_d = nc.declare_dram_parameter("u", [bpc, S, DIN], f32, isOutput=False)
    w_d = nc.declare_dram_parameter("W", [1, DIN, O], f32, isOutput=False)
    cb_d = nc.declare_dram_parameter("CB", [128, 289], bf16, isOutput=False)
    cf_d = nc.declare_dram_parameter("CF", [128, 640], f32, isOutput=False)
    out_d = nc.declare_dram_parameter("out", [bpc, NC_CAP, DC], f32, isOutput=True)

    with ExitStack() as ctx:
        tc = ctx.enter_context(tile.TileContext(nc))
        const = ctx.enter_context(tc.tile_pool(name="const", bufs=1))
        sb_u = ctx.enter_context(tc.tile_pool(name="sb_u", bufs=1))
        sb_t = ctx.enter_context(tc.tile_pool(name="sb_t", bufs=1))
        sb_c = ctx.enter_context(tc.tile_pool(name="sb_c", bufs=1))
        work = ctx.enter_context(tc.tile_pool(name="work", bufs=2))
        ps_tp = ctx.enter_context(tc.tile_pool(name="ps_tp", bufs=2, space="PSUM"))
        ps_z = ctx.enter_context(tc.tile_pool(name="ps_z", bufs=3, space="PSUM"))
        ps_q = ctx.enter_context(tc.tile_pool(name="ps_q", bufs=3, space="PSUM"))

        # ---- constants (cb first; cf/W DMAs are issued after group 0's u so
        # the first batches don't share HBM bandwidth with them) ----
        cb = const.tile([128, 289], bf16, tag="cb")
        nc.sync.dma_start(cb[:], cb_d[:])
        cf = const.tile([128, 640], f32, tag="cf")
        ident = cb[:, 0:128]
        c0 = cb[:, 128:256]
        psel = cb[:, 256:288]
        ones_col = cb[:, 288:289]
        mask3 = cf[:, 0:512].rearrange("p (ot c) -> p ot c", ot=4)
        onesf_row = cf[0:1, 512:640]
        epsc = const.tile([1, 1], f32, tag="epsc")
        nc.vector.memset(epsc[:], EPS)

        # ---- W load, cast, transpose (emitted after prologue(0)) ----
        wf = const.tile([128, 2, 512], f32, tag="wf")
        wbt = const.tile([128, 2, 512], bf16, tag="wbt")
        wtb = [const.tile([128, 256], bf16, tag=f"wtb{ot}", name=f"wtb{ot}")
               for ot in range(4)]

        def w_prep():
            nc.sync.dma_start(cf[:], cf_d[:])
            nc.sync.dma_start(wf[:], w_d[0].rearrange("(it p) o -> p it o", p=128))
            nc.vector.tensor_copy(wbt[:], wf[:])
            for ot in range(4):
                ptw = ps_tp.tile([128, 2, 512], bf16, tag="tp")
                for it in range(2):
                    nc.tensor.transpose(
                        ptw[:, 0, 128 * it:128 * (it + 1)],
                        wbt[:, it, 128 * ot:128 * (ot + 1)], ident)
                nc.scalar.copy(wtb[ot][:], ptw[:, 0, 0:256])

        ub = [None] * bpc
        uts = [None] * bpc
        cT = {}   # g -> [128, 4(st), NB, 16] bf16 tile (c for next iter)

        ufs = {}

        def prologue_dma(g):
            for l in range(NB):
                b = NB * g + l
                # batches 0/1 get a private staging tag so batch 0's DMA does
                # not share HBM bandwidth 4 ways (faster pipeline head)
                if b < 2:
                    uf = work.tile([128, 4, 256], f32, tag="uf0", bufs=2)
                else:
                    uf = work.tile([128, 4, 256], f32, tag="uf", bufs=6)
                nc.sync.dma_start(uf[:], u_d[b].rearrange("(st p) i -> p st i", p=128))
                ufs[b] = uf

        def prologue_tp(g, lo, hi):
            # casts emitted here (not at DMA issue) so DMA-paced casts sit
            # behind the latency-critical routing ops in the engine streams
            for l in range(lo, hi):
                b = NB * g + l
                t = sb_u.tile([128, 4, 256], bf16, tag=f"ub{b}")
                if b % 2 == 0:
                    nc.vector.tensor_copy(t[:], ufs[b][:])
                else:
                    nc.scalar.copy(t[:], ufs[b][:])
                ub[b] = t
            for l in range(lo, hi):
                b = NB * g + l
                ptp = ps_tp.tile([128, 2, 512], bf16, tag="tp")
                for it in range(2):
                    for st in range(4):
                        nc.tensor.transpose(
                            ptp[:, it, 128 * st:128 * (st + 1)],
                            ub[b][:, st, 128 * it:128 * (it + 1)], ident)
                t = sb_t.tile([128, 2, 512], bf16, tag=f"ut{b}")
                if b % 3 == 0:
                    nc.scalar.copy(t[:], ptp[:])
                else:
                    nc.vector.tensor_copy(t[:], ptp[:])
                uts[b] = t

        def crhs(g, k, st, l):
            if k == 0:
                return c0[:, 16 * l:16 * (l + 1)]
            return cT[g][:, st, l, :]

        rstate = {}

        def routing_A(g, k):
            # one PSUM bank per group-iter for ZT/norm/bcast/G (regions reused
            # sequentially; Tile subtile deps order the writers/readers)
            zfat = ps_z.tile([128, 3, 128], f32, tag="z")
            pz = zfat[:, 0:2, :]
            # ---- ZT[i,(l,n)] = sum_s u[s,i]*c[s,(l,n)] ----
            for it in range(2):
                for l in range(NB):
                    for st in range(4):
                        nc.tensor.matmul(
                            pz[:, it, 16 * l:16 * (l + 1)],
                            ub[NB * g + l][:, st, 128 * it:128 * (it + 1)],
                            crhs(g, k, st, l),
                            start=(st == 0), stop=(st == 3))
            zb = work.tile([128, 2, 128], bf16, tag="zb", bufs=4)
            nc.vector.tensor_copy(zb[:], pz[:])
            # ---- P[o,(l,n)] = sum_i W[i,o]*ZT[i,(l,n)];  V = P*mask ----
            pp = ps_q.tile([128, 4, 128], f32, tag="q")
            for ot in range(4):
                for it in range(2):
                    nc.tensor.matmul(pp[:, ot, :],
                                     wbt[:, it, 128 * ot:128 * (ot + 1)],
                                     zb[:, it, :], start=(it == 0), stop=(it == 1))
            vb = work.tile([128, 4, 128], bf16, tag="vb", bufs=4)
            nc.vector.tensor_mul(vb[:], pp[:], mask3)
            # ---- G[i,(l,n)] = sum_o W[i,o]*V[o,(l,n)] (raw; scaled in B) ----
            if k < ROUT - 1:
                pg = zfat[:, 0:2, :]
                for it in range(2):
                    for ot in range(4):
                        nc.tensor.matmul(pg[:, it, :],
                                         wtb[ot][:, 128 * it:128 * (it + 1)],
                                         vb[:, ot, :], start=(ot == 0), stop=(ot == 3))
            vsq = work.tile([128, 4, 128], bf16, tag="vsq", bufs=3)
            nc.scalar.activation(vsq[:], vb[:], AF.Square)
            # ---- |V|^2 col sums (accumulate the 4 o-tiles into [1,128]) ----
            pn = zfat[0:1, 2, :]
            for ot in range(4):
                nc.tensor.matmul(pn, ones_col, vsq[:, ot, :],
                                 start=(ot == 0), stop=(ot == 3))
            # rsqrt = exp(-0.5*ln(x+eps)): Ln/Exp/Copy/Square share one Act
            # function table set, so no per-iter act-table reloads
            sq = work.tile([1, 128], f32, tag="sq")
            nc.scalar.activation(sq[:], pn, AF.Ln, bias=epsc[:])
            rsn = work.tile([1, 128], f32, tag="rsn")
            nc.scalar.activation(rsn[:], sq[:], AF.Exp, scale=-0.5)
            rstate[g] = (zfat, vb, rsn)

        def routing_B(g, k):
            zfat, vb, rsn = rstate[g]
            pbc = zfat[:, 2, :]
            nc.tensor.matmul(pbc, onesf_row, rsn[:], start=True, stop=True)
            if k < ROUT - 1:
                snsb = work.tile([128, 128], f32, tag="snsb", bufs=3)
                nc.scalar.copy(snsb[:], pbc)
                gb = work.tile([128, 2, 128], bf16, tag="gb", bufs=3)
                nc.vector.tensor_mul(
                    gb[:], zfat[:, 0:2, :],
                    snsb[:].unsqueeze(1).broadcast_to((128, 2, 128)))
                # ---- bT[s,(l,n)] = sum_i u[s,i]*G[i,(l,n)] (= b*rsqrt) ----
                pbt = ps_q.tile([128, 4, 128], f32, tag="q")
                for st in range(4):
                    for l in range(NB):
                        for it in range(2):
                            nc.tensor.matmul(
                                pbt[:, st, 16 * l:16 * (l + 1)],
                                uts[NB * g + l][:, it, 128 * st:128 * (st + 1)],
                                gb[:, it, 16 * l:16 * (l + 1)],
                                start=(it == 0), stop=(it == 1))
                # softmax over n (16-blocks), scale-free (c ~ 16*softmax)
                e = work.tile([128, 4, NB, 16], f32, tag="e", bufs=4)
                nc.scalar.activation(
                    e[:], pbt[:].rearrange("p st (l n) -> p st l n", n=16), AF.Exp)
                d8 = work.tile([128, 4, NB, 8], f32, tag="d8")
                nc.vector.tensor_add(d8[:], e[:, :, :, 0:8], e[:, :, :, 8:16])
                d4 = work.tile([128, 4, NB, 4], f32, tag="d4")
                nc.vector.tensor_add(d4[:], d8[:, :, :, 0:4], d8[:, :, :, 4:8])
                d2 = work.tile([128, 4, NB, 2], f32, tag="d2")
                nc.vector.tensor_add(d2[:], d4[:, :, :, 0:2], d4[:, :, :, 2:4])
                d1 = work.tile([128, 4, NB, 1], f32, tag="d1")
                nc.vector.tensor_add(d1[:], d2[:, :, :, 0:1], d2[:, :, :, 1:2])
                rr = work.tile([128, 4, NB, 1], f32, tag="rr")
                nc.vector.reciprocal(rr[:], d1[:])
                ct = sb_c.tile([128, 4, NB, 16], bf16, tag=f"c{g}_{k % 2}")
                nc.vector.tensor_mul(
                    ct[:], e[:], rr[:].broadcast_to((128, 4, NB, 16)))
                cT[g] = ct
            else:
                # ---- final: vs = V*rsqrt; extract [(l,n), d] via psel ----
                vs = work.tile([128, 4, 128], bf16, tag="vs")
                nc.vector.tensor_mul(
                    vs[:], vb[:], pbc.unsqueeze(1).broadcast_to((128, 4, 128)))
                pout = ps_q.tile([128, 4, 128], f32, tag="q")
                for ot in range(4):
                    nc.tensor.matmul(pout[:, 0, 0:32], vs[:, ot, :], psel,
                                     start=(ot == 0), stop=(ot == 3))
                osb = work.tile([128, 32], f32, tag="osb")
                nc.scalar.copy(osb[:], pout[:, 0, 0:32])
                nc.sync.dma_start(
                    out_d[NB * g:NB * (g + 1)].rearrange("b n d -> (b n) d"),
                    osb[:])

        # software-pipelined emission: prologue(g) at step g, iter k at step g+1+k
        def final_dummy_out():
            # phase-ablation builds still must write the declared output
            osb = work.tile([128, 32], f32, tag="osb")
            nc.vector.tensor_copy(osb[:], ub[0][0:128, 0, 0:32])
            for g in range(ng):
                nc.sync.dma_start(
                    out_d[NB * g:NB * (g + 1)].rearrange("b n d -> (b n) d"),
                    osb[:])

        for _rep in range(repeat):
            if phase == "dma":
                for g in range(ng):
                    for l in range(NB):
                        b = NB * g + l
                        uf = work.tile([128, 4, 256], f32, tag="uf", bufs=3)
                        nc.sync.dma_start(
                            uf[:], u_d[b].rearrange("(st p) i -> p st i", p=128))
                        t = sb_u.tile([128, 4, 256], bf16, tag=f"ub{b}")
                        if b % 3 == 0:
                            nc.vector.tensor_copy(t[:], uf[:])
                        else:
                            nc.scalar.copy(t[:], uf[:])
                        ub[b] = t
                final_dummy_out()
                continue
            if phase == "prologue":
                for g in range(ng):
                    prologue_dma(g)
                    prologue_tp(g, 0, NB)
                final_dummy_out()
                continue
            # deeper pipeline: prologue(g) at step g; A(g,k) at step g+1+2k;
            # B(g,k) at step g+2+2k — a full step of other groups' work sits
            # between each A and its B, hiding the norm/softmax chains
            for step in range(ng + 2 * ROUT + 1):
                # DMAs for this step's new group issue first (SP stream)
                if step < ng:
                    prologue_dma(step)
                    if step == 0:
                        w_prep()
                # B phases' inputs are from the previous step: all ready, so
                # they go first in every engine stream
                for g in range(ng):
                    p = step - g - 2
                    if p >= 0 and p % 2 == 0 and p // 2 < ROUT:
                        routing_B(g, p // 2)
                # interleave the new group's casts+transposes in chunks between
                # the A phases so engines don't head-of-line block on DMA pace
                achunks = []
                for g in range(ng):
                    p = step - g - 1
                    if p >= 0 and p % 2 == 0 and p // 2 < ROUT:
                        achunks.append((g, p // 2))
                tp_cuts = [0, 2, 4, 6, NB] if step < ng else []
                for idx in range(max(len(achunks), len(tp_cuts) - 1)):
                    if idx < len(achunks):
                        routing_A(*achunks[idx])
                    if step < ng and idx < len(tp_cuts) - 1:
                        prologue_tp(step, tp_cuts[idx], tp_cuts[idx + 1])
    return nc


def kernel(u_vecs, W):
    global LAST_RESULTS
    try:
        _install_compile_patch()
        from concourse.bass_utils import run_bass_kernel_spmd
        nc = _build_bass()
        cb, cf = _const_blocks()
        u = np.ascontiguousarray(u_vecs, dtype=np.float32)
        w = np.ascontiguousarray(W, dtype=np.float32)
        in_maps = [
            {"u": u[c * BPC:(c + 1) * BPC], "W": w, "CB": cb, "CF": cf}
            for c in range(NCORES)
        ]
        res = run_bass_kernel_spmd(nc, in_maps, core_ids=list(range(NCORES)))
        LAST_RESULTS = res
        out = np.concatenate([res.results[c]["out"] for c in range(NCORES)], axis=0)
        return out.astype(np.float32)
    except Exception as ex:
        import traceback
        traceback.print_exc(file=sys.stderr)
        sys.stderr.write(f"[kernel.py] bass path failed ({ex!r}); numpy fallback\n")
        return _kernel_numpy(u_vecs, W)
